# revision 15
# baseline (speedup 1.0000x reference)
"""Trainium2 Bass kernel for nn_CNNBlock_3770981285925.

Reference computation (B=4096, CH=512, NF=128, L=4, S=5):
    x = x @ W1.T + b1                          # [B, NF]
    for i in range(L):
        dt = softplus(u[i]) / S
        5x:  h = relu(x @ A_i.T + bA_i);  x += dt * (h @ A_i) * alpha_i
        5x:  ff = relu(x @ Bw_i.T + Bb_i) @ Cw_i.T
             mat = triu_scatter(ff); mat -= mat.T     # skew [B, NF, NF]
             x += dt * einsum('bjk,bk->bj', mat, x)

Device strategy (pure data parallel, 8 cores x 512 samples):
  State kept transposed in SBUF: xT [NF=128 partitions, 512 batch].
  The skew step uses  mat[b] = sum_f h[b,f] * K_f  with fixed skew
  matrices K_f = scatter(Cw[:,f]) - scatter(Cw[:,f]).T, so
      yT = sum_f (dt*K_f).T @ (xT * h[f, :])
  i.e. 128 PSUM-accumulated bf16 matmuls per step. The row-broadcast
  h[f,:] is materialized on the PE via a K=1 matmul (ones ⊗ h-row), and
  the product xT*hB runs on the vector engine straight out of PSUM.
  All other linear algebra (W1, grad steps, h pre-relu) is exact fp32.
"""

import sys
from contextlib import ExitStack

import numpy as np

sys.path.insert(0, "/opt/trn_rl_repo")

import ml_dtypes  # noqa: E402

import concourse.bass as bass  # noqa: E402
import concourse.tile as tile  # noqa: E402
from concourse import bacc, mybir  # noqa: E402

B, CH, NF, L, S = 4096, 512, 128, 4, 5
N_CORES = 8
BC = B // N_CORES  # 512 samples per core
IU0, IU1 = np.triu_indices(NF, 1)

F32 = mybir.dt.float32
BF16 = mybir.dt.bfloat16
AF = mybir.ActivationFunctionType
ALU = mybir.AluOpType


def prep_weights(W1, b1, A, bA, alpha, Bw, Bb, Cw, u):
    """Host-side weight preparation (all numpy, small tensors)."""
    W1 = np.asarray(W1, np.float32)
    A = np.asarray(A, np.float32)
    Bw = np.asarray(Bw, np.float32)
    Cw = np.asarray(Cw, np.float32)
    u64 = np.asarray(u, np.float64)
    dts = (np.logaddexp(0.0, u64) / S).astype(np.float32)  # softplus(u)/S

    # lhsT for x1T = W1 @ xT0 : lhsT[ch, f] = W1[f, ch], chunked over ch
    lhsT_W1 = np.ascontiguousarray(W1.T.reshape(CH // NF, NF, NF))  # [4,128c,128f]

    lhsT_A1 = np.stack([A[i].T for i in range(L)])  # [L, c, f] for h2T = A xT
    lhsT_A2 = np.ascontiguousarray(A)  # [L, f, c] for gT = A.T... (h2 @ A)^T
    lhsT_Bw = np.stack([Bw[i].T for i in range(L)])  # [L, k, f] for hT = Bw xT

    # K_all[k, i, f, j] = dts[i] * K_f^{(i)}[j, k]
    K_all = np.zeros((NF, L, NF, NF), np.float32)
    for i in range(L):
        Km = np.zeros((NF, NF, NF), np.float32)  # [f, j, k]
        Km[:, IU0, IU1] = Cw[i].T
        Km = Km - np.swapaxes(Km, 1, 2)
        # lhsT[k, j] = dt * K_f[j, k]
        K_all[:, i, :, :] = dts[i] * np.transpose(Km, (2, 0, 1))
    K_all = K_all.astype(ml_dtypes.bfloat16)

    # biases / per-partition scales, packed [NF, 1+3L]
    # cols: 0: b1 | 1..L: bA | L+1..2L: Bb | 2L+1..3L: dts*alpha
    bias_all = np.zeros((NF, 1 + 3 * L), np.float32)
    bias_all[:, 0] = np.asarray(b1, np.float32)
    for i in range(L):
        bias_all[:, 1 + i] = np.asarray(bA[i], np.float32)
        bias_all[:, 1 + L + i] = np.asarray(Bb[i], np.float32)
        bias_all[:, 1 + 2 * L + i] = dts[i] * np.asarray(alpha[i], np.float32)

    # SEL_full[p, r, :] = ones iff p % 32 == r (broadcast-row selector)
    sel = np.zeros((NF, 32, NF), np.float32)
    for p in range(NF):
        sel[p, p % 32, :] = 1.0
    sel = sel.astype(ml_dtypes.bfloat16)

    return {
        "lhsT_W1": np.ascontiguousarray(lhsT_W1),
        "lhsT_A1": np.ascontiguousarray(lhsT_A1),
        "lhsT_A2": np.ascontiguousarray(lhsT_A2),
        "lhsT_Bw": np.ascontiguousarray(lhsT_Bw),
        "K_all": np.ascontiguousarray(K_all),
        "bias_all": bias_all,
        "sel": sel,
    }


def build_program():
    """Build + compile the per-core Bass program (identical on all cores)."""
    nc = bacc.Bacc("TRN2", target_bir_lowering=False, debug=False)

    xT0_d = nc.dram_tensor("xT0", [CH, BC], F32, kind="ExternalInput")
    W1_d = nc.dram_tensor("lhsT_W1", [CH // NF, NF, NF], F32, kind="ExternalInput")
    A1_d = nc.dram_tensor("lhsT_A1", [L, NF, NF], F32, kind="ExternalInput")
    A2_d = nc.dram_tensor("lhsT_A2", [L, NF, NF], F32, kind="ExternalInput")
    Bw_d = nc.dram_tensor("lhsT_Bw", [L, NF, NF], F32, kind="ExternalInput")
    K_d = nc.dram_tensor("K_all", [NF, L, NF, NF], BF16, kind="ExternalInput")
    bias_d = nc.dram_tensor("bias_all", [NF, 1 + 3 * L], F32, kind="ExternalInput")
    sel_d = nc.dram_tensor("sel", [NF, 32, NF], BF16, kind="ExternalInput")
    out_d = nc.dram_tensor("out", [NF, BC], F32, kind="ExternalOutput")

    with tile.TileContext(nc) as tc, ExitStack() as ctx:
        consts = ctx.enter_context(tc.tile_pool(name="consts", bufs=1))
        state = ctx.enter_context(tc.tile_pool(name="state", bufs=1))
        work = ctx.enter_context(tc.tile_pool(name="work", bufs=3))
        pwork = ctx.enter_context(tc.tile_pool(name="pwork", bufs=6))
        evwork = ctx.enter_context(tc.tile_pool(name="evwork", bufs=8))
        psum_y = ctx.enter_context(tc.tile_pool(name="psum_y", bufs=1, space="PSUM"))
        psum_hb = ctx.enter_context(tc.tile_pool(name="psum_hb", bufs=3, space="PSUM"))
        psum_m = ctx.enter_context(tc.tile_pool(name="psum_m", bufs=1, space="PSUM"))

        # ---- load constants ----
        W1_s = consts.tile([NF, CH // NF, NF], F32)  # [c-part, chunk, f]
        for c in range(CH // NF):
            nc.sync.dma_start(out=W1_s[:, c, :], in_=W1_d.ap()[c])
        A1_s = consts.tile([NF, L, NF], F32)
        A2_s = consts.tile([NF, L, NF], F32)
        Bw_s = consts.tile([NF, L, NF], F32)
        for i in range(L):
            nc.sync.dma_start(out=A1_s[:, i, :], in_=A1_d.ap()[i])
            nc.sync.dma_start(out=A2_s[:, i, :], in_=A2_d.ap()[i])
            nc.sync.dma_start(out=Bw_s[:, i, :], in_=Bw_d.ap()[i])
        bias_s = consts.tile([NF, 1 + 3 * L], F32)
        nc.sync.dma_start(out=bias_s, in_=bias_d.ap())
        K_s = consts.tile([NF, L, NF, NF], BF16)
        for i in range(L):
            nc.sync.dma_start(out=K_s[:, i, :, :], in_=K_d.ap()[:, i, :, :])
        # SEL_full[p, r, :] = ones if p % 32 == r else 0 — K=32 stationary that
        # broadcasts row r of a 32-partition group to all 128 output partitions.
        sel_s = consts.tile([NF, 32, NF], BF16)
        nc.sync.dma_start(out=sel_s, in_=sel_d.ap())

        xT0_s = state.tile([NF, CH // NF, BC], F32)
        for c in range(CH // NF):
            nc.sync.dma_start(out=xT0_s[:, c, :], in_=xT0_d.ap()[bass.ts(c, NF), :])

        xT = state.tile([NF, BC], F32)

        # ---- x = x @ W1.T + b1  (as xT = W1 xT0 + b1) ----
        x1_ps = psum_m.tile([NF, BC], F32, tag="mm")
        for c in range(CH // NF):
            nc.tensor.matmul(
                x1_ps,
                lhsT=W1_s[:, c, :],
                rhs=xT0_s[:, c, :],
                start=(c == 0),
                stop=(c == CH // NF - 1),
            )
        nc.scalar.activation(xT, x1_ps, AF.Identity, bias=bias_s[:, 0:1])

        for i in range(L):
            # ---- gradient sub-steps ----
            for s in range(S):
                h2_ps = psum_m.tile([NF, BC], F32, tag="mm")
                nc.tensor.matmul(
                    h2_ps, lhsT=A1_s[:, i, :], rhs=xT, start=True, stop=True
                )
                h2 = work.tile([NF, BC], F32, tag="h2")
                nc.scalar.activation(
                    h2, h2_ps, AF.Relu, bias=bias_s[:, 1 + i : 2 + i]
                )
                g_ps = psum_m.tile([NF, BC], F32, tag="mm")
                nc.tensor.matmul(
                    g_ps, lhsT=A2_s[:, i, :], rhs=h2, start=True, stop=True
                )
                # xT = (g_ps * (dt*alpha)) + xT
                nc.vector.scalar_tensor_tensor(
                    out=xT,
                    in0=g_ps,
                    scalar=bias_s[:, 1 + 2 * L + i : 2 + 2 * L + i],
                    in1=xT,
                    op0=ALU.mult,
                    op1=ALU.add,
                )

            # ---- skew (sph) sub-steps ----
            # product-path assignment per pair (8-periodic):
            #   'a' = DVE multiply straight out of PSUM (fp32 1x)
            #   'b' = ACT evac PSUM->SBUF bf16, DVE bf16 2x multiply
            #   'g' = ACT evac PSUM->SBUF fp32, GPSIMD multiply
            PATTERN = [
                "b", "a", "b", "b", "a", "b", "b", "a",
                "b", "b", "a", "b", "b", "a", "b", "b",
            ]
            for s in range(S):
                h_ps = psum_m.tile([NF, BC], F32, tag="mm")
                nc.tensor.matmul(
                    h_ps, lhsT=Bw_s[:, i, :], rhs=xT, start=True, stop=True
                )
                h_bf = work.tile([NF, BC], BF16, tag="hbf")
                nc.scalar.activation(
                    h_bf, h_ps, AF.Relu, bias=bias_s[:, 1 + L + i : 2 + L + i]
                )
                x_bf = work.tile([NF, BC], BF16, tag="xbf")
                nc.scalar.copy(x_bf, xT)

                y_ps = psum_y.tile([NF, BC], F32)
                x_b = xT[:, None, :].broadcast_to([NF, 2, BC])
                x_bfb = x_bf[:, None, :].broadcast_to([NF, 2, BC])
                for r in range(64):
                    # pair of f's {r, 64+r}: two row-group-disjoint SEL
                    # broadcast matmuls run concurrently on the PE
                    fs = (r, 64 + r)
                    hb2 = psum_hb.tile([NF, 2, BC], F32, tag="hb")
                    for t, f in enumerate(fs):
                        g, rr = divmod(f, 32)
                        nc.tensor.matmul(
                            hb2[:, t, :],
                            lhsT=sel_s[32 * g : 32 * g + 32, rr, :],
                            rhs=h_bf[32 * g : 32 * g + 32, :],
                            start=True,
                            stop=True,
                            tile_position=(32 * g, 0),
                        )
                    p2 = pwork.tile([NF, 2, BC], BF16, tag="p")
                    path = PATTERN[r % 16]
                    if path == "a":
                        nc.vector.tensor_tensor(
                            out=p2, in0=x_b, in1=hb2, op=ALU.mult
                        )
                    elif path == "b":
                        hb_bf = evwork.tile([NF, 2, BC], BF16, tag="hbbf")
                        nc.scalar.copy(hb_bf, hb2)
                        nc.vector.tensor_tensor(
                            out=p2, in0=x_bfb, in1=hb_bf, op=ALU.mult
                        )
                    else:
                        raise AssertionError(path)
                    for t, f in enumerate(fs):
                        nc.tensor.matmul(
                            y_ps,
                            lhsT=K_s[:, i, f, :],
                            rhs=p2[:, t, :],
                            start=(r == 0 and t == 0),
                            stop=(r == 63 and t == 1),
                        )
                # xT = xT + y_ps   (dt folded into K)
                nc.vector.tensor_tensor(out=xT, in0=xT, in1=y_ps, op=ALU.add)

        nc.sync.dma_start(out=out_d.ap(), in_=xT)

    nc.compile()
    return nc


_CACHE = {}


def _get_program():
    if "nc" not in _CACHE:
        _CACHE["nc"] = build_program()
    return _CACHE["nc"]


def _install_ntff_hook():
    """This image's antenv lacks axon_hooks; synthesize it so trace=True works."""
    import types

    if "antenv.axon_hooks" in sys.modules:
        return
    mod = types.ModuleType("antenv.axon_hooks")
    mod._hook = None
    mod.set_axon_ntff_profile_hook = lambda h: setattr(mod, "_hook", h)
    mod.get_axon_ntff_profile_hook = lambda: mod._hook
    sys.modules["antenv.axon_hooks"] = mod
    try:
        from trn_agent_boot.trn_boot import _ntff_profile_via_ctypes

        mod._hook = _ntff_profile_via_ctypes("/opt/axon/libaxon_pjrt.so")
    except Exception:
        pass


def run_sharded(inputs, trace=False, trace_kwargs=None):
    """Run the SPMD kernel on 8 cores. Returns (full_output, BassKernelResults)."""
    from concourse import bass_utils

    if trace:
        _install_ntff_hook()
        # artifact upload has no backing store in this container
        bass_utils.upload_artifacts = lambda tmpdir: tmpdir

    x = np.asarray(inputs["x"], np.float32)
    w = prep_weights(
        inputs["W1"], inputs["b1"], inputs["A"], inputs["bA"], inputs["alpha"],
        inputs["Bw"], inputs["Bb"], inputs["Cw"], inputs["u"],
    )
    nc = _get_program()

    in_maps = []
    for c in range(N_CORES):
        shard = x[c * BC : (c + 1) * BC]  # [BC, CH]
        m = {"xT0": np.ascontiguousarray(shard.T)}
        m.update(w)
        in_maps.append(m)

    kw = dict(trace_kwargs or {})
    res = bass_utils.run_bass_kernel_spmd(
        nc, in_maps, core_ids=list(range(N_CORES)), trace=trace, **kw
    )
    outs = [res.results[c]["out"].T for c in range(N_CORES)]  # [BC, NF] each
    full = np.concatenate(outs, axis=0).astype(np.float32)
    return full, res


def kernel(**inputs) -> np.ndarray:
    out, _ = run_sharded(inputs, trace=False)
    return out


# revision 17
# speedup vs baseline: 1.0995x; 1.0995x over previous
"""Trainium2 Bass kernel for nn_CNNBlock_3770981285925.

Reference computation (B=4096, CH=512, NF=128, L=4, S=5):
    x = x @ W1.T + b1                          # [B, NF]
    for i in range(L):
        dt = softplus(u[i]) / S
        5x:  h = relu(x @ A_i.T + bA_i);  x += dt * (h @ A_i) * alpha_i
        5x:  ff = relu(x @ Bw_i.T + Bb_i) @ Cw_i.T
             mat = triu_scatter(ff); mat -= mat.T     # skew [B, NF, NF]
             x += dt * einsum('bjk,bk->bj', mat, x)

Device strategy (pure data parallel, 8 cores x 512 samples):
  State kept transposed in SBUF: xT [NF=128 partitions, 512 batch].
  The skew step uses  mat[b] = sum_f h[b,f] * K_f  with fixed skew
  matrices K_f = scatter(Cw[:,f]) - scatter(Cw[:,f]).T, so
      yT = sum_f (dt*K_f).T @ (xT * h[f, :])
  i.e. 128 PSUM-accumulated bf16 matmuls per step. The row-broadcast
  h[f,:] is materialized on the PE via a K=1 matmul (ones ⊗ h-row), and
  the product xT*hB runs on the vector engine straight out of PSUM.
  All other linear algebra (W1, grad steps, h pre-relu) is exact fp32.
"""

import sys
from contextlib import ExitStack

import numpy as np

sys.path.insert(0, "/opt/trn_rl_repo")

import ml_dtypes  # noqa: E402

import concourse.bass as bass  # noqa: E402
import concourse.tile as tile  # noqa: E402
from concourse import bacc, mybir  # noqa: E402

B, CH, NF, L, S = 4096, 512, 128, 4, 5
N_CORES = 8
BC = B // N_CORES  # 512 samples per core
IU0, IU1 = np.triu_indices(NF, 1)

F32 = mybir.dt.float32
BF16 = mybir.dt.bfloat16
AF = mybir.ActivationFunctionType
ALU = mybir.AluOpType


def prep_weights(W1, b1, A, bA, alpha, Bw, Bb, Cw, u):
    """Host-side weight preparation (all numpy, small tensors)."""
    W1 = np.asarray(W1, np.float32)
    A = np.asarray(A, np.float32)
    Bw = np.asarray(Bw, np.float32)
    Cw = np.asarray(Cw, np.float32)
    u64 = np.asarray(u, np.float64)
    dts = (np.logaddexp(0.0, u64) / S).astype(np.float32)  # softplus(u)/S

    # lhsT for x1T = W1 @ xT0 : lhsT[ch, f] = W1[f, ch], chunked over ch
    lhsT_W1 = np.ascontiguousarray(W1.T.reshape(CH // NF, NF, NF))  # [4,128c,128f]

    lhsT_A1 = np.stack([A[i].T for i in range(L)])  # [L, c, f] for h2T = A xT
    lhsT_A2 = np.ascontiguousarray(A)  # [L, f, c] for gT = A.T... (h2 @ A)^T
    lhsT_Bw = np.stack([Bw[i].T for i in range(L)])  # [L, k, f] for hT = Bw xT

    # K_all[k, i, f, j] = dts[i] * K_f^{(i)}[j, k]
    K_all = np.zeros((NF, L, NF, NF), np.float32)
    for i in range(L):
        Km = np.zeros((NF, NF, NF), np.float32)  # [f, j, k]
        Km[:, IU0, IU1] = Cw[i].T
        Km = Km - np.swapaxes(Km, 1, 2)
        # lhsT[k, j] = dt * K_f[j, k]
        K_all[:, i, :, :] = dts[i] * np.transpose(Km, (2, 0, 1))
    K_all = K_all.astype(ml_dtypes.bfloat16)

    # biases / per-partition scales, packed [NF, 1+3L]
    # cols: 0: b1 | 1..L: bA | L+1..2L: Bb | 2L+1..3L: dts*alpha
    bias_all = np.zeros((NF, 1 + 3 * L), np.float32)
    bias_all[:, 0] = np.asarray(b1, np.float32)
    for i in range(L):
        bias_all[:, 1 + i] = np.asarray(bA[i], np.float32)
        bias_all[:, 1 + L + i] = np.asarray(Bb[i], np.float32)
        bias_all[:, 1 + 2 * L + i] = dts[i] * np.asarray(alpha[i], np.float32)

    # SEL_full[p, r, :] = ones iff p % 32 == r (broadcast-row selector)
    sel = np.zeros((NF, 32, NF), np.float32)
    for p in range(NF):
        sel[p, p % 32, :] = 1.0
    sel = sel.astype(ml_dtypes.bfloat16)

    return {
        "lhsT_W1": np.ascontiguousarray(lhsT_W1),
        "lhsT_A1": np.ascontiguousarray(lhsT_A1),
        "lhsT_A2": np.ascontiguousarray(lhsT_A2),
        "lhsT_Bw": np.ascontiguousarray(lhsT_Bw),
        "K_all": np.ascontiguousarray(K_all),
        "bias_all": bias_all,
        "sel": sel,
    }


def build_program():
    """Build + compile the per-core Bass program (identical on all cores)."""
    nc = bacc.Bacc("TRN2", target_bir_lowering=False, debug=False)

    xT0_d = nc.dram_tensor("xT0", [CH, BC], F32, kind="ExternalInput")
    W1_d = nc.dram_tensor("lhsT_W1", [CH // NF, NF, NF], F32, kind="ExternalInput")
    A1_d = nc.dram_tensor("lhsT_A1", [L, NF, NF], F32, kind="ExternalInput")
    A2_d = nc.dram_tensor("lhsT_A2", [L, NF, NF], F32, kind="ExternalInput")
    Bw_d = nc.dram_tensor("lhsT_Bw", [L, NF, NF], F32, kind="ExternalInput")
    K_d = nc.dram_tensor("K_all", [NF, L, NF, NF], BF16, kind="ExternalInput")
    bias_d = nc.dram_tensor("bias_all", [NF, 1 + 3 * L], F32, kind="ExternalInput")
    sel_d = nc.dram_tensor("sel", [NF, 32, NF], BF16, kind="ExternalInput")
    out_d = nc.dram_tensor("out", [NF, BC], F32, kind="ExternalOutput")

    with tile.TileContext(nc) as tc, ExitStack() as ctx:
        consts = ctx.enter_context(tc.tile_pool(name="consts", bufs=1))
        state = ctx.enter_context(tc.tile_pool(name="state", bufs=1))
        work = ctx.enter_context(tc.tile_pool(name="work", bufs=3))
        pwork = ctx.enter_context(tc.tile_pool(name="pwork", bufs=8))
        evwork = ctx.enter_context(tc.tile_pool(name="evwork", bufs=4))
        psum_y = ctx.enter_context(tc.tile_pool(name="psum_y", bufs=1, space="PSUM"))
        psum_hb = ctx.enter_context(tc.tile_pool(name="psum_hb", bufs=3, space="PSUM"))
        psum_m = ctx.enter_context(tc.tile_pool(name="psum_m", bufs=1, space="PSUM"))

        # ---- load constants ----
        W1_s = consts.tile([NF, CH // NF, NF], F32)  # [c-part, chunk, f]
        for c in range(CH // NF):
            nc.sync.dma_start(out=W1_s[:, c, :], in_=W1_d.ap()[c])
        A1_s = consts.tile([NF, L, NF], F32)
        A2_s = consts.tile([NF, L, NF], F32)
        Bw_s = consts.tile([NF, L, NF], F32)
        for i in range(L):
            nc.sync.dma_start(out=A1_s[:, i, :], in_=A1_d.ap()[i])
            nc.sync.dma_start(out=A2_s[:, i, :], in_=A2_d.ap()[i])
            nc.sync.dma_start(out=Bw_s[:, i, :], in_=Bw_d.ap()[i])
        bias_s = consts.tile([NF, 1 + 3 * L], F32)
        nc.sync.dma_start(out=bias_s, in_=bias_d.ap())
        K_s = consts.tile([NF, L, NF, NF], BF16)
        for i in range(L):
            nc.sync.dma_start(out=K_s[:, i, :, :], in_=K_d.ap()[:, i, :, :])
        # SEL_full[p, r, :] = ones if p % 32 == r else 0 — K=32 stationary that
        # broadcasts row r of a 32-partition group to all 128 output partitions.
        sel_s = consts.tile([NF, 32, NF], BF16)
        nc.sync.dma_start(out=sel_s, in_=sel_d.ap())

        xT = state.tile([NF, BC], F32)

        # ---- x = x @ W1.T + b1  (as xT = W1 xT0 + b1) ----
        with tc.tile_pool(name="xload", bufs=1) as xload:
            xT0_s = xload.tile([NF, CH // NF, BC], F32)
            for c in range(CH // NF):
                nc.sync.dma_start(
                    out=xT0_s[:, c, :], in_=xT0_d.ap()[bass.ts(c, NF), :]
                )
            x1_ps = psum_m.tile([NF, BC], F32, tag="mm")
            for c in range(CH // NF):
                nc.tensor.matmul(
                    x1_ps,
                    lhsT=W1_s[:, c, :],
                    rhs=xT0_s[:, c, :],
                    start=(c == 0),
                    stop=(c == CH // NF - 1),
                )
            nc.scalar.activation(xT, x1_ps, AF.Identity, bias=bias_s[:, 0:1])

        for i in range(L):
            # ---- gradient sub-steps ----
            for s in range(S):
                h2_ps = psum_m.tile([NF, BC], F32, tag="mm")
                nc.tensor.matmul(
                    h2_ps, lhsT=A1_s[:, i, :], rhs=xT, start=True, stop=True
                )
                h2 = work.tile([NF, BC], F32, tag="h2")
                nc.scalar.activation(
                    h2, h2_ps, AF.Relu, bias=bias_s[:, 1 + i : 2 + i]
                )
                g_ps = psum_m.tile([NF, BC], F32, tag="mm")
                nc.tensor.matmul(
                    g_ps, lhsT=A2_s[:, i, :], rhs=h2, start=True, stop=True
                )
                # xT = (g_ps * (dt*alpha)) + xT
                nc.vector.scalar_tensor_tensor(
                    out=xT,
                    in0=g_ps,
                    scalar=bias_s[:, 1 + 2 * L + i : 2 + 2 * L + i],
                    in1=xT,
                    op0=ALU.mult,
                    op1=ALU.add,
                )

            # ---- skew (sph) sub-steps ----
            # product-path assignment per pair (8-periodic):
            #   'a' = DVE multiply straight out of PSUM (fp32 1x)
            #   'b' = ACT evac PSUM->SBUF bf16, DVE bf16 2x multiply
            #   'g' = ACT evac PSUM->SBUF fp32, GPSIMD multiply
            PATTERN = [
                "b", "a", "b", "g", "b", "a", "b", "b",
                "a", "b", "b", "g", "a", "b", "b", "a",
            ]
            for s in range(S):
                h_ps = psum_m.tile([NF, BC], F32, tag="mm")
                nc.tensor.matmul(
                    h_ps, lhsT=Bw_s[:, i, :], rhs=xT, start=True, stop=True
                )
                h_bf = work.tile([NF, BC], BF16, tag="hbf")
                nc.scalar.activation(
                    h_bf, h_ps, AF.Relu, bias=bias_s[:, 1 + L + i : 2 + L + i]
                )
                x_bf = work.tile([NF, BC], BF16, tag="xbf")
                nc.scalar.copy(x_bf, xT)

                y_ps = psum_y.tile([NF, BC], F32)
                x_b = xT[:, None, :].broadcast_to([NF, 2, BC])
                x_bfb = x_bf[:, None, :].broadcast_to([NF, 2, BC])
                # software pipeline: K-matmuls for pair r are emitted LAG
                # pairs after its producer chain, so the PE never reaches a
                # K-matmul before its product tile is ready.
                LAG = 4
                p_tiles = {}
                for r in range(64 + LAG):
                    if r < 64:
                        fs = (r, 64 + r)
                        hb2 = psum_hb.tile([NF, 2, BC], F32, tag="hb")
                        for t, f in enumerate(fs):
                            g, rr = divmod(f, 32)
                            nc.tensor.matmul(
                                hb2[:, t, :],
                                lhsT=sel_s[32 * g : 32 * g + 32, rr, :],
                                rhs=h_bf[32 * g : 32 * g + 32, :],
                                start=True,
                                stop=True,
                                tile_position=(32 * g, 0),
                            )
                        p2 = pwork.tile([NF, 2, BC], BF16, tag="p")
                        path = PATTERN[r % 16]
                        if path == "a":
                            nc.vector.tensor_tensor(
                                out=p2, in0=x_b, in1=hb2, op=ALU.mult
                            )
                        elif path == "b":
                            hb_bf = evwork.tile([NF, 2, BC], BF16, tag="hbbf")
                            nc.scalar.copy(hb_bf, hb2)
                            nc.vector.tensor_tensor(
                                out=p2, in0=x_bfb, in1=hb_bf, op=ALU.mult
                            )
                        else:
                            hb_sb = evwork.tile([NF, 2, BC], F32, tag="hbsb")
                            nc.scalar.copy(hb_sb, hb2)
                            nc.gpsimd.tensor_tensor(
                                out=p2, in0=x_b, in1=hb_sb, op=ALU.mult
                            )
                        p_tiles[r] = p2
                    rk = r - LAG
                    if rk >= 0:
                        p2k = p_tiles.pop(rk)
                        for t, f in enumerate((rk, 64 + rk)):
                            nc.tensor.matmul(
                                y_ps,
                                lhsT=K_s[:, i, f, :],
                                rhs=p2k[:, t, :],
                                start=(rk == 0 and t == 0),
                                stop=(rk == 63 and t == 1),
                            )
                # xT = xT + y_ps   (dt folded into K)
                nc.vector.tensor_tensor(out=xT, in0=xT, in1=y_ps, op=ALU.add)

        nc.sync.dma_start(out=out_d.ap(), in_=xT)

    nc.compile()
    return nc


_CACHE = {}


def _get_program():
    if "nc" not in _CACHE:
        _CACHE["nc"] = build_program()
    return _CACHE["nc"]


def _install_ntff_hook():
    """This image's antenv lacks axon_hooks; synthesize it so trace=True works."""
    import types

    if "antenv.axon_hooks" in sys.modules:
        return
    mod = types.ModuleType("antenv.axon_hooks")
    mod._hook = None
    mod.set_axon_ntff_profile_hook = lambda h: setattr(mod, "_hook", h)
    mod.get_axon_ntff_profile_hook = lambda: mod._hook
    sys.modules["antenv.axon_hooks"] = mod
    try:
        from trn_agent_boot.trn_boot import _ntff_profile_via_ctypes

        mod._hook = _ntff_profile_via_ctypes("/opt/axon/libaxon_pjrt.so")
    except Exception:
        pass


def run_sharded(inputs, trace=False, trace_kwargs=None):
    """Run the SPMD kernel on 8 cores. Returns (full_output, BassKernelResults)."""
    from concourse import bass_utils

    if trace:
        _install_ntff_hook()
        # artifact upload has no backing store in this container
        bass_utils.upload_artifacts = lambda tmpdir: tmpdir

    x = np.asarray(inputs["x"], np.float32)
    w = prep_weights(
        inputs["W1"], inputs["b1"], inputs["A"], inputs["bA"], inputs["alpha"],
        inputs["Bw"], inputs["Bb"], inputs["Cw"], inputs["u"],
    )
    nc = _get_program()

    in_maps = []
    for c in range(N_CORES):
        shard = x[c * BC : (c + 1) * BC]  # [BC, CH]
        m = {"xT0": np.ascontiguousarray(shard.T)}
        m.update(w)
        in_maps.append(m)

    kw = dict(trace_kwargs or {})
    res = bass_utils.run_bass_kernel_spmd(
        nc, in_maps, core_ids=list(range(N_CORES)), trace=trace, **kw
    )
    outs = [res.results[c]["out"].T for c in range(N_CORES)]  # [BC, NF] each
    full = np.concatenate(outs, axis=0).astype(np.float32)
    return full, res


def kernel(**inputs) -> np.ndarray:
    out, _ = run_sharded(inputs, trace=False)
    return out
